# revision 32
# baseline (speedup 1.0000x reference)
"""Trainium2 Bass kernel for BertSelfAttention variant (logsigmoid-fused QK attention).

Reference computation (B=2, S=2048, D=1024, H=16, dh=64):
    q = split_heads(hidden @ Wq + bq)
    k = split_heads(hidden @ Wk + bk)
    k1 = logsigmoid(q) + q + k ; k2 = logsigmoid(k1)
    scores = -(q @ k2^T) / 8 ; probs = softmax(scores) ; ctx = probs @ q

With kk := -k2 >= 0, exactly kk = ln(1 + e^{-q-k}(1 + e^{-q})), so the
elementwise chain needs only two projections (Wq and host-fused Wq+Wk):
    ev2 = exp(-(qp + bq))        = e^{-q}    # ACT, straight from qp PSUM
    eu  = exp(-(ap + bq + bk))   = e^{-q-k}  # ACT, from ap PSUM
    ut  = eu * ev2 + eu                       # DVE mul + add
    kk  = ln(1 + ut)                          # ACT Ln, batched [128,2048]
All activations (Exp + Ln) live in the natural_log_exp_and_others table,
pre-loaded once via an explicit InstLoadActFuncSet -- no mid-kernel
activation-table swaps.

Sharding: 8 cores = 2 (batch) x 4 (head groups of 4 heads / 256 cols of Wq,Wk).
Each core computes its [2048, 256] slice of the output; host reassembles.

Device layout is fully transposed so no matmul ever needs a transposed operand:
    qT, kkT [256(dout), 2048(s)]  from  ht = hidden[b].T  (host-side transpose)
    scoresT[kpos, q] = kkT_head^T @ qT_head    (row-packed head pairs, K=64)
    expT = Exp(scoresT / 8)                    (one [128,1024] ACT op per chunk)
    ctx_aug[65, q] = sum_kpos v_aug[kpos,65]^T @ expT[kpos, q]
        v_aug = [v | 1] -> row 64 accumulates the softmax denominator.
Raw transposed ctx + denominators go back to DRAM; the host divides and
re-transposes while unsharding (no device-side finalize transposes).

Schedule (v2): both halves' projections run upfront as back-to-back matmul
chains (PE stays continuously busy -> p-state ramps to 2.4GHz early).  The
projection (qp|ap) pairs live in the same rotating PSUM slots the score
tiles use later (tag sp, 2x[128,1024]); v_aug PE-transposes are batched
4-wide into one-bank tpv tiles (tags cA/cB, which become the ctx accumulators
during streams).  In the stream phase the prev-stream ctx drains ride between
score matmuls as sp-independent PE filler, so the PE keeps long busy streaks
while ACT stays saturated on the score exps.

Matmuls run in bf16; the softmax denominator path and output stay fp32.
"""

import numpy as np

B, S, D = 2, 2048, 1024
H, DH = 16, 64
NCORES = 8
HG = 4  # head-group count (tensor parallel)
CPG = (H // HG) * DH  # cols per group = 256
NDT = D // 128  # 8 din tiles
NSC = S // 512  # 4 s-chunks (projection) == 4 q-chunks (attention)
NKC = S // 128  # 16 kpos chunks

MM_DTYPE = "bf16"  # "bf16" | "f32r" | "f32"

_compiled = None
LAST_RESULT = None


def _build():
    from contextlib import ExitStack

    import concourse.bacc as bacc
    import concourse.mybir as mybir
    import concourse.tile as tile

    from concourse.alu_op_type import AluOpType as AluOp

    f32 = mybir.dt.float32
    mmdt = {
        "bf16": mybir.dt.bfloat16,
        "f32r": mybir.dt.float32r,
        "f32": mybir.dt.float32,
    }[MM_DTYPE]
    AF = mybir.ActivationFunctionType

    nc = bacc.Bacc("TRN2", target_bir_lowering=False, debug=False)
    ht = nc.dram_tensor("ht", [D, S], mmdt, kind="ExternalInput").ap()
    # host-fused weight wall: [Wq | Wq+Wk] per row, one load per tile
    wall = nc.dram_tensor("wall", [D, 2 * CPG], mmdt, kind="ExternalInput").ap()
    # smalls cols: pbq[0:2] nbq[2:4] nbqk[4:6] ones[6:22]
    smalls = nc.dram_tensor("smalls", [128, 22], f32, kind="ExternalInput").ap()
    # identity for the v transposes, in the matmul dtype (I64 stacked twice)
    idb = nc.dram_tensor("idb", [128, 64], mmdt, kind="ExternalInput").ap()
    # 65 rows per head: 64 raw ctx rows + the softmax-denominator row, so one
    # finalize needs a single DMA (no separate dens tensor / tiny transfers).
    out = nc.dram_tensor("out", [4 * 65, S], f32, kind="ExternalOutput").ap()

    # natural_log_exp_and_others: serves every Exp and Ln in the kernel with
    # a single activation-table load (pre-placed below; the compiler's
    # per-function chooser would otherwise bounce between the exp-only and
    # ln-only tables, 1283ns per swap).
    from concourse.hw_specs import get_activation_tables

    nlx_id = list(get_activation_tables(nc.m.arch)).index(
        "natural_log_exp_and_others"
    )

    with tile.TileContext(nc) as tc, ExitStack() as ctx:
        const = ctx.enter_context(tc.tile_pool(name="const", bufs=1))
        big = ctx.enter_context(tc.tile_pool(name="big", bufs=1))
        sb = ctx.enter_context(tc.tile_pool(name="sb", bufs=2))
        # One PSUM pool, exactly 8 banks:
        #   sp  bufs=2 x [128,1536] f32 = 6 banks (proj pairs, then score tiles)
        #   cA  bufs=1 x 1 bank          (tpv transposes, then ctx head rr=0)
        #   cB  bufs=1 x 1 bank          (tpv transposes, then ctx head rr=1)
        # ctx tags can be single-buffered because a stream's finalize is
        # always emitted before the next stream's first drain write.
        ps = ctx.enter_context(tc.tile_pool(name="ps", bufs=1, space="PSUM"))
        csp = ctx.enter_context(tc.tile_pool(name="csp", bufs=4))

        nc.scalar.add_instruction(
            mybir.InstLoadActFuncSet(act_func_set_id=nlx_id)
        )

        sm = const.tile([128, 22], f32, tag="smalls")
        nc.sync.dma_start(sm[:], smalls[:])
        pbq_t = sm[:, 0:2]
        nbq_t = sm[:, 2:4]
        nbqk_t = sm[:, 4:6]
        ones_t = sm[:, 6:22]
        idb_t = const.tile([128, 64], mmdt, tag="idb")
        nc.sync.dma_start(idb_t[:], idb[:])

        # Inputs needed first come first, interleaved across both HWDGE
        # rings (SP + ACT): per din tile j, its weight chunk plus the sc=0
        # hidden chunk; then the remaining hidden chunks sc-major so chunk
        # sc is complete across all j before chunk sc+1 lands.
        rr_ring = [nc.sync, nc.scalar]
        ring_i = 0

        def ring():
            nonlocal ring_i
            ring_i += 1
            return rr_ring[ring_i % 2]

        wqs, was, hts = [], [], []
        for j in range(NDT):
            t_ = big.tile([128, S], mmdt, tag=f"ht{j}", name=f"hts{j}")
            hts.append(t_)
        for j in range(NDT):
            w = const.tile([128, 2 * CPG], mmdt, tag=f"w{j}", name=f"walls{j}")
            ring().dma_start(w[:], wall[j * 128 : (j + 1) * 128, :])
            wqs.append(w[:, 0:CPG])
            was.append(w[:, CPG : 2 * CPG])
            ring().dma_start(
                hts[j][:, 0:512], ht[j * 128 : (j + 1) * 128, 0:512]
            )
        # remaining hidden columns as two half-width transfers per tile
        # (1.5KiB rows: few descriptors, good DMA bw) with all the first
        # halves ahead of the second, so chunk sc=1 lands early
        for j in range(NDT):
            ring().dma_start(
                hts[j][:, 512:1280], ht[j * 128 : (j + 1) * 128, 512:1280]
            )
        for j in range(NDT):
            ring().dma_start(
                hts[j][:, 1280:S], ht[j * 128 : (j + 1) * 128, 1280:S]
            )

        q_sb = [big.tile([128, S], mmdt, tag=f"q{t}", name=f"q{t}") for t in range(2)]
        kk_sb = [big.tile([128, S], mmdt, tag=f"kk{t}", name=f"kk{t}") for t in range(2)]
        ut = [big.tile([128, S], f32, tag=f"ut{t}", name=f"ut{t}") for t in range(2)]
        vaug = [
            big.tile([128, NKC * 65], mmdt, tag=f"v{h}", name=f"v{h}") for h in range(4)
        ]
        # three stream-sized exp buffers; drains index [128,1024] sub-slices.
        # Three (not two) so stream i+1's exps land in a different buffer
        # than the one stream i's drains are still reading (i+1 vs i-1).
        etb = [
            big.tile([128, NKC * 1024], mmdt, tag=f"et{r}", name=f"et{r}")
            for r in range(3)
        ]

        # ---------- emission helpers ----------
        def emit_vaug_ones(t):
            for rr in range(2):
                vv = vaug[2 * t + rr][:].rearrange("p (c w) -> p c w", w=65)
                nc.vector.tensor_copy(
                    vv[:, :, 64:65], ones_t.rearrange("p (c w) -> p c w", w=1)
                )

        def emit_proj_chunk(t, sc):
            """Two matmul chains for one [dout-half, 512] chunk + elementwise."""
            ssl = slice(sc * 512, (sc + 1) * 512)
            pr = ps.tile([128, 1536], f32, tag="sp", name="pr", bufs=2)
            for j in range(NDT):
                nc.tensor.matmul(
                    pr[:, 0:512],
                    lhsT=wqs[j][:, t * 128 : (t + 1) * 128],
                    rhs=hts[j][:, ssl],
                    start=(j == 0),
                    stop=(j == NDT - 1),
                )
            for j in range(NDT):
                nc.tensor.matmul(
                    pr[:, 512:1024],
                    lhsT=was[j][:, t * 128 : (t + 1) * 128],
                    rhs=hts[j][:, ssl],
                    start=(j == 0),
                    stop=(j == NDT - 1),
                )
            ev2 = sb.tile([128, 512], f32, tag="ev2")
            nc.scalar.activation(
                ev2[:], pr[:, 0:512], AF.Exp, bias=nbq_t[:, t : t + 1], scale=-1.0
            )
            eu = sb.tile([128, 512], f32, tag="eu")
            nc.scalar.activation(
                eu[:], pr[:, 512:1024], AF.Exp, bias=nbqk_t[:, t : t + 1], scale=-1.0
            )
            nc.vector.tensor_scalar_add(q_sb[t][:, ssl], pr[:, 0:512], pbq_t[:, t : t + 1])
            # ut = (ev2 + 1) * eu  ==  e^{-q-k}(1 + e^{-q})  in one DVE op
            nc.vector.scalar_tensor_tensor(
                ut[t][:, ssl], ev2[:], 1.0, eu[:], AluOp.add, AluOp.mult
            )

        def emit_kk(t):
            nc.scalar.activation(kk_sb[t][:], ut[t][:], AF.Ln, bias=1.0, scale=1.0)

        def emit_vaug_chunk(t, sc, tag):
            """PE-transpose one [64,512] q chunk per head of half t, batched:
            4 [128,64] transposes into one tpv tile, one DVE copy out."""
            for rr in range(2):
                lh = 2 * t + rr
                hsl = slice(rr * 64, rr * 64 + 64)
                tpv = ps.tile(
                    [128, 256], mmdt, tag=tag[rr], name="tpv", bufs=1
                )
                for jj in range(4):
                    j = 4 * sc + jj
                    nc.tensor.transpose(
                        tpv[:, jj * 64 : (jj + 1) * 64],
                        q_sb[t][hsl, j * 128 : (j + 1) * 128],
                        idb_t[hsl, 0:64],
                    )
                vv = vaug[lh][:].rearrange("p (c w) -> p c w", w=65)
                nc.vector.tensor_copy(
                    vv[:, 4 * sc : 4 * sc + 4, 0:64],
                    tpv[:].rearrange("p (c w) -> p c w", w=64),
                )

        def emit_drain_chunk(prev_state, kc_rev, immediate=False):
            qc_p, t_p, ebuf_p, ctxs_p = prev_state
            for rr in range(2):
                nc.tensor.matmul(
                    ctxs_p[rr][:],
                    lhsT=vaug[2 * t_p + rr][:, kc_rev * 65 : kc_rev * 65 + 65],
                    rhs=ebuf_p[:, kc_rev * 1024 + rr * 512 : kc_rev * 1024 + rr * 512 + 512],
                    start=(kc_rev == (0 if immediate else NKC - 1)),
                    stop=(kc_rev == (NKC - 1 if immediate else 0)),
                )

        def emit_finalize(prev_state):
            qc_p, t_p, ebuf_p, ctxs_p = prev_state
            qsl_p = slice(qc_p * 512, (qc_p + 1) * 512)
            for rr in range(2):
                lh = 2 * t_p + rr
                cs = csp.tile([128, 512], f32, tag="cs")
                nc.vector.tensor_copy(cs[0:65, :], ctxs_p[rr][:])
                # one 65-row DMA (ctx + denominator) on the sync ring, which
                # is idle through the stream phase
                nc.sync.dma_start(out[lh * 65 : lh * 65 + 65, qsl_p], cs[0:65, :])

        # ---------- schedule ----------
        # Phase P: all projections, t-major (t=0 fully first).  Once kk(0)
        # exists, stream 0's first eight exp units weave between the t=1
        # projection chunks, filling ACT's otherwise-idle projection window
        # with early score exps.  v_aug transposes for a chunk are emitted
        # one iteration later, so the PE never waits on the DVE q-add that
        # produces their input.
        emit_vaug_ones(0)
        emit_vaug_ones(1)

        streams = [(qc, t) for t in range(2) for qc in range(NSC)]
        NS = len(streams)
        states = [None] * NS
        NG = NKC // 3  # 5 full 3-kc groups; kc=15 rides alone as a tail unit
        NU = 2 * NG + 1  # 11 emission units per stream

        def emit_score(s, kc, rr, dst):
            qc, t = streams[s]
            nc.tensor.matmul(
                dst,
                lhsT=kk_sb[t][rr * 64 : rr * 64 + 64, kc * 128 : (kc + 1) * 128],
                rhs=q_sb[t][rr * 64 : rr * 64 + 64, qc * 512 : (qc + 1) * 512],
                start=True,
                stop=True,
            )

        def emit_unit(s, u):
            if u == 0:
                # ctx tiles are allocated lazily at the stream's first drain
                # (mid next stream), so they never overlap the tpv transposes
                # sharing the cA/cB banks during the projection phase.
                states[s] = (streams[s][0], streams[s][1], etb[s % 3], None)
            ebuf = states[s][2]
            sp = ps.tile([128, 1536], f32, tag="sp", name="sp", bufs=2)
            if u == NU - 1:  # tail: kc = 15 alone, 1024-wide exp
                kc = NKC - 1
                emit_score(s, kc, 0, sp[:, 0:512])
                emit_score(s, kc, 1, sp[:, 512:1024])
                nc.scalar.activation(
                    ebuf[:, kc * 1024 : (kc + 1) * 1024],
                    sp[:, 0:1024],
                    AF.Exp,
                    scale=0.125,
                )
                return
            g, half = u // 2, u % 2
            if half == 0:
                emit_score(s, 3 * g, 0, sp[:, 0:512])
                emit_score(s, 3 * g, 1, sp[:, 512:1024])
                emit_score(s, 3 * g + 1, 0, sp[:, 1024:1536])
                base = 3 * g * 1024
            else:
                emit_score(s, 3 * g + 1, 1, sp[:, 0:512])
                emit_score(s, 3 * g + 2, 0, sp[:, 512:1024])
                emit_score(s, 3 * g + 2, 1, sp[:, 1024:1536])
                base = 3 * g * 1024 + 1536
            nc.scalar.activation(
                ebuf[:, base : base + 1536], sp[:], AF.Exp, scale=0.125
            )

        def emit_drain_for(n):
            s, kc = n // NKC, n % NKC
            if kc == 0:
                qc, t, ebuf, _ = states[s]
                states[s] = (
                    qc,
                    t,
                    ebuf,
                    [
                        ps.tile([65, 512], f32, tag="cA", name="ctxA", bufs=1),
                        ps.tile([65, 512], f32, tag="cB", name="ctxB", bufs=1),
                    ],
                )
            emit_drain_chunk(states[s], kc, immediate=True)
            if kc == NKC - 1:
                emit_finalize(states[s])

        def unit_exped(kc):
            if kc == NKC - 1:
                return NU - 1
            g, r = kc // 3, kc % 3
            return 2 * g if r == 0 else 2 * g + 1


        total_units = NS * NU
        due = [[] for _ in range(total_units + 1)]
        for s in range(NS):
            for kc in range(NKC):
                m = s * NKC + kc
                if s < NS - 2:
                    # drain during the next stream, spread one pair per unit
                    gu = (s + 1) * NU + (kc * NU) // NKC
                elif s == NS - 2:
                    # compress into the final stream's first half so its
                    # finalize frees the ctx banks before the final stream's
                    # own drains need them
                    gu = (s + 1) * NU + (kc * 6) // NKC
                else:
                    # final stream: drains start right after the previous
                    # stream's finalize (same unit, emitted just before)
                    gu = s * NU + max(unit_exped(kc) + 2, 5 + (kc * 5) // NKC)
                due[min(gu, total_units)].append(m)

        order = [(sc, t) for t in range(2) for sc in range(NSC)]
        for it, (sc, t) in enumerate(order):
            if it == 4:
                emit_kk(0)
            emit_proj_chunk(t, sc)
            if it >= 1:
                psc, pt = order[it - 1]
                emit_vaug_chunk(pt, psc, tag=("cA", "cB"))
            if it >= 4:
                emit_unit(0, 2 * (it - 4))
                emit_unit(0, 2 * (it - 4) + 1)
        emit_kk(1)
        emit_vaug_chunk(1, 3, tag=("cA", "cB"))

        # Phase S: continue the flat unit pipeline (stream 0's first eight
        # units were already emitted during the projection weave).
        for U in range(total_units):
            s, u = U // NU, U % NU
            if not (s == 0 and u < 8):
                emit_unit(s, u)
            for m in due[U]:
                emit_drain_for(m)
        for m in due[total_units]:
            emit_drain_for(m)

    nc.compile()
    return nc


def kernel(hidden_states, attention_mask, Wq, bq, Wk, bk):
    global _compiled, LAST_RESULT
    hs = np.asarray(hidden_states, dtype=np.float32)
    am = np.asarray(attention_mask)
    Wq = np.asarray(Wq, dtype=np.float32)
    Wk = np.asarray(Wk, dtype=np.float32)
    bq = np.asarray(bq, dtype=np.float32)
    bk = np.asarray(bk, dtype=np.float32)

    if _compiled is None:
        _compiled = _build()
    nc = _compiled

    from concourse.bass_utils import run_bass_kernel_spmd

    if MM_DTYPE == "bf16":
        import ml_dtypes

        def to_mmdt(x):
            return np.ascontiguousarray(
                np.asarray(x, np.float32).astype(ml_dtypes.bfloat16)
            )

    elif MM_DTYPE == "f32r":

        def to_mmdt(x):
            # fp32r = 1s/8e/11m (top 20 bits of fp32), round-to-nearest-even
            b = np.ascontiguousarray(x, dtype=np.float32).view(np.uint32)
            lsb = (b >> np.uint32(12)) & np.uint32(1)
            r = (b + np.uint32(0x7FF) + lsb) & np.uint32(0xFFFFF000)
            return r.view(np.float32)

    else:

        def to_mmdt(x):
            return np.ascontiguousarray(x, dtype=np.float32)

    idb = to_mmdt(np.tile(np.eye(64, dtype=np.float32), (2, 1)))
    in_maps = []
    for c in range(NCORES):
        b, g = c // HG, c % HG
        cols = slice(g * CPG, (g + 1) * CPG)
        bq_s = bq[cols].reshape(2, 128).T
        bk_s = bk[cols].reshape(2, 128).T
        smalls = np.concatenate(
            [bq_s, -bq_s, -(bq_s + bk_s), np.ones((128, 16), np.float32)],
            axis=1,
        ).astype(np.float32)
        in_maps.append(
            {
                "ht": to_mmdt(hs[b].T),
                "wall": to_mmdt(
                    np.concatenate(
                        [Wq[:, cols], Wq[:, cols] + Wk[:, cols]],
                        axis=1,
                    )
                ),
                "smalls": np.ascontiguousarray(smalls),
                "idb": idb,
            }
        )

    res = run_bass_kernel_spmd(nc, in_maps, list(range(NCORES)))
    LAST_RESULT = res

    outp = np.empty((B, S, H * DH), dtype=np.float32)
    for c in range(NCORES):
        b, g = c // HG, c % HG
        raw = res.results[c]["out"].reshape(4, 65, S)  # per head: 64 ctx + den
        ctxT = raw[:, 0:64, :] / raw[:, 64:65, :]
        outp[b, :, g * CPG : (g + 1) * CPG] = ctxT.reshape(CPG, S).T

    # attention_mask==0 masks whole query rows -> uniform probs -> ctx row is
    # the mean of q over all key positions. Never triggers for all-ones masks.
    if (am == 0).any():
        for b in range(B):
            rows = np.nonzero(am[b] == 0)[0]
            if rows.size:
                q_full = hs[b] @ Wq + bq
                outp[b, rows, :] = q_full.mean(axis=0)
    return outp


# revision 33
# speedup vs baseline: 1.1977x; 1.1977x over previous
"""Trainium2 Bass kernel for BertSelfAttention variant (logsigmoid-fused QK attention).

Reference computation (B=2, S=2048, D=1024, H=16, dh=64):
    q = split_heads(hidden @ Wq + bq)
    k = split_heads(hidden @ Wk + bk)
    k1 = logsigmoid(q) + q + k ; k2 = logsigmoid(k1)
    scores = -(q @ k2^T) / 8 ; probs = softmax(scores) ; ctx = probs @ q

With kk := -k2 >= 0, exactly kk = ln(1 + e^{-q-k}(1 + e^{-q})), so the
elementwise chain needs only two projections (Wq and host-fused Wq+Wk):
    ev2 = exp(-(qp + bq))        = e^{-q}    # ACT, straight from qp PSUM
    eu  = exp(-(ap + bq + bk))   = e^{-q-k}  # ACT, from ap PSUM
    ut  = eu * ev2 + eu                       # DVE mul + add
    kk  = ln(1 + ut)                          # ACT Ln, batched [128,2048]
All activations (Exp + Ln) live in the natural_log_exp_and_others table,
pre-loaded once via an explicit InstLoadActFuncSet -- no mid-kernel
activation-table swaps.

Sharding: 8 cores = 2 (batch) x 4 (head groups of 4 heads / 256 cols of Wq,Wk).
Each core computes its [2048, 256] slice of the output; host reassembles.

Device layout is fully transposed so no matmul ever needs a transposed operand:
    qT, kkT [256(dout), 2048(s)]  from  ht = hidden[b].T  (host-side transpose)
    scoresT[kpos, q] = kkT_head^T @ qT_head    (row-packed head pairs, K=64)
    expT = Exp(scoresT / 8)                    (one [128,1024] ACT op per chunk)
    ctx_aug[65, q] = sum_kpos v_aug[kpos,65]^T @ expT[kpos, q]
        v_aug = [v | 1] -> row 64 accumulates the softmax denominator.
Raw transposed ctx + denominators go back to DRAM; the host divides and
re-transposes while unsharding (no device-side finalize transposes).

Schedule (v2): both halves' projections run upfront as back-to-back matmul
chains (PE stays continuously busy -> p-state ramps to 2.4GHz early).  The
projection (qp|ap) pairs live in the same rotating PSUM slots the score
tiles use later (tag sp, 2x[128,1024]); v_aug PE-transposes are batched
4-wide into one-bank tpv tiles (tags cA/cB, which become the ctx accumulators
during streams).  In the stream phase the prev-stream ctx drains ride between
score matmuls as sp-independent PE filler, so the PE keeps long busy streaks
while ACT stays saturated on the score exps.

Matmuls run in bf16; the softmax denominator path and output stay fp32.
"""

import numpy as np

B, S, D = 2, 2048, 1024
H, DH = 16, 64
NCORES = 8
HG = 4  # head-group count (tensor parallel)
CPG = (H // HG) * DH  # cols per group = 256
NDT = D // 128  # 8 din tiles
NSC = S // 512  # 4 s-chunks (projection) == 4 q-chunks (attention)
NKC = S // 128  # 16 kpos chunks

MM_DTYPE = "bf16"  # "bf16" | "f32r" | "f32"

_compiled = None
LAST_RESULT = None


def _build():
    from contextlib import ExitStack

    import concourse.bacc as bacc
    import concourse.mybir as mybir
    import concourse.tile as tile

    from concourse.alu_op_type import AluOpType as AluOp

    f32 = mybir.dt.float32
    mmdt = {
        "bf16": mybir.dt.bfloat16,
        "f32r": mybir.dt.float32r,
        "f32": mybir.dt.float32,
    }[MM_DTYPE]
    AF = mybir.ActivationFunctionType

    nc = bacc.Bacc("TRN2", target_bir_lowering=False, debug=False)
    ht = nc.dram_tensor("ht", [D, S], mmdt, kind="ExternalInput").ap()
    # host-fused weight wall: [Wq | Wq+Wk] per row, one load per tile
    wall = nc.dram_tensor("wall", [D, 2 * CPG], mmdt, kind="ExternalInput").ap()
    # smalls cols: pbq[0:2] nbq[2:4] nbqk[4:6] ones[6:22]
    smalls = nc.dram_tensor("smalls", [128, 22], f32, kind="ExternalInput").ap()
    # identity for the v transposes, in the matmul dtype (I64 stacked twice)
    idb = nc.dram_tensor("idb", [128, 64], mmdt, kind="ExternalInput").ap()
    # 65 rows per head: 64 raw ctx rows + the softmax-denominator row, so one
    # finalize needs a single DMA (no separate dens tensor / tiny transfers).
    out = nc.dram_tensor("out", [4 * 65, S], f32, kind="ExternalOutput").ap()

    # natural_log_exp_and_others: serves every Exp and Ln in the kernel with
    # a single activation-table load (pre-placed below; the compiler's
    # per-function chooser would otherwise bounce between the exp-only and
    # ln-only tables, 1283ns per swap).
    from concourse.hw_specs import get_activation_tables

    nlx_id = list(get_activation_tables(nc.m.arch)).index(
        "natural_log_exp_and_others"
    )

    with tile.TileContext(nc) as tc, ExitStack() as ctx:
        const = ctx.enter_context(tc.tile_pool(name="const", bufs=1))
        big = ctx.enter_context(tc.tile_pool(name="big", bufs=1))
        sb = ctx.enter_context(tc.tile_pool(name="sb", bufs=2))
        # One PSUM pool, exactly 8 banks:
        #   sp  bufs=2 x [128,1536] f32 = 6 banks (proj pairs, then score tiles)
        #   cA  bufs=1 x 1 bank          (tpv transposes, then ctx head rr=0)
        #   cB  bufs=1 x 1 bank          (tpv transposes, then ctx head rr=1)
        # ctx tags can be single-buffered because a stream's finalize is
        # always emitted before the next stream's first drain write.
        ps = ctx.enter_context(tc.tile_pool(name="ps", bufs=1, space="PSUM"))
        csp = ctx.enter_context(tc.tile_pool(name="csp", bufs=4))

        nc.scalar.add_instruction(
            mybir.InstLoadActFuncSet(act_func_set_id=nlx_id)
        )

        sm = const.tile([128, 22], f32, tag="smalls")
        nc.sync.dma_start(sm[:], smalls[:])
        pbq_t = sm[:, 0:2]
        nbq_t = sm[:, 2:4]
        nbqk_t = sm[:, 4:6]
        ones_t = sm[:, 6:22]
        idb_t = const.tile([128, 64], mmdt, tag="idb")
        nc.sync.dma_start(idb_t[:], idb[:])

        # Inputs needed first come first, interleaved across both HWDGE
        # rings (SP + ACT): per din tile j, its weight chunk plus the sc=0
        # hidden chunk; then the remaining hidden chunks sc-major so chunk
        # sc is complete across all j before chunk sc+1 lands.
        rr_ring = [nc.sync, nc.scalar]
        ring_i = 0

        def ring():
            nonlocal ring_i
            ring_i += 1
            return rr_ring[ring_i % 2]

        wqs, was, hts = [], [], []
        for j in range(NDT):
            t_ = big.tile([128, S], mmdt, tag=f"ht{j}", name=f"hts{j}")
            hts.append(t_)
        for j in range(NDT):
            w = const.tile([128, 2 * CPG], mmdt, tag=f"w{j}", name=f"walls{j}")
            ring().dma_start(w[:], wall[j * 128 : (j + 1) * 128, :])
            wqs.append(w[:, 0:CPG])
            was.append(w[:, CPG : 2 * CPG])
            ring().dma_start(
                hts[j][:, 0:512], ht[j * 128 : (j + 1) * 128, 0:512]
            )
        # remaining hidden columns as two half-width transfers per tile
        # (1.5KiB rows: few descriptors, good DMA bw) with all the first
        # halves ahead of the second, so chunk sc=1 lands early
        for j in range(NDT):
            ring().dma_start(
                hts[j][:, 512:1280], ht[j * 128 : (j + 1) * 128, 512:1280]
            )
        for j in range(NDT):
            ring().dma_start(
                hts[j][:, 1280:S], ht[j * 128 : (j + 1) * 128, 1280:S]
            )

        q_sb = [big.tile([128, S], mmdt, tag=f"q{t}", name=f"q{t}") for t in range(2)]
        kk_sb = [big.tile([128, S], mmdt, tag=f"kk{t}", name=f"kk{t}") for t in range(2)]
        ut = [big.tile([128, S], f32, tag=f"ut{t}", name=f"ut{t}") for t in range(2)]
        vaug = [
            big.tile([128, NKC * 65], mmdt, tag=f"v{h}", name=f"v{h}") for h in range(4)
        ]
        # three stream-sized exp buffers; drains index [128,1024] sub-slices.
        # Three (not two) so stream i+1's exps land in a different buffer
        # than the one stream i's drains are still reading (i+1 vs i-1).
        etb = [
            big.tile([128, NKC * 1024], mmdt, tag=f"et{r}", name=f"et{r}")
            for r in range(3)
        ]

        # ---------- emission helpers ----------
        def emit_vaug_ones(t):
            for rr in range(2):
                vv = vaug[2 * t + rr][:].rearrange("p (c w) -> p c w", w=65)
                nc.vector.tensor_copy(
                    vv[:, :, 64:65], ones_t.rearrange("p (c w) -> p c w", w=1)
                )

        def emit_proj_chunk(t, sc):
            """Two matmul chains for one [dout-half, 512] chunk + elementwise."""
            ssl = slice(sc * 512, (sc + 1) * 512)
            pr = ps.tile([128, 1536], f32, tag="sp", name="pr", bufs=2)
            for j in range(NDT):
                nc.tensor.matmul(
                    pr[:, 0:512],
                    lhsT=wqs[j][:, t * 128 : (t + 1) * 128],
                    rhs=hts[j][:, ssl],
                    start=(j == 0),
                    stop=(j == NDT - 1),
                )
            for j in range(NDT):
                nc.tensor.matmul(
                    pr[:, 512:1024],
                    lhsT=was[j][:, t * 128 : (t + 1) * 128],
                    rhs=hts[j][:, ssl],
                    start=(j == 0),
                    stop=(j == NDT - 1),
                )
            ev2 = sb.tile([128, 512], f32, tag="ev2")
            nc.scalar.activation(
                ev2[:], pr[:, 0:512], AF.Exp, bias=nbq_t[:, t : t + 1], scale=-1.0
            )
            eu = sb.tile([128, 512], f32, tag="eu")
            nc.scalar.activation(
                eu[:], pr[:, 512:1024], AF.Exp, bias=nbqk_t[:, t : t + 1], scale=-1.0
            )
            nc.vector.tensor_scalar_add(q_sb[t][:, ssl], pr[:, 0:512], pbq_t[:, t : t + 1])
            # ut = (ev2 + 1) * eu  ==  e^{-q-k}(1 + e^{-q})  in one DVE op
            nc.vector.scalar_tensor_tensor(
                ut[t][:, ssl], ev2[:], 1.0, eu[:], AluOp.add, AluOp.mult
            )

        def emit_kk(t):
            nc.scalar.activation(kk_sb[t][:], ut[t][:], AF.Ln, bias=1.0, scale=1.0)

        def emit_vaug_chunk(t, sc, tag):
            """PE-transpose one [64,512] q chunk per head of half t, batched:
            4 [128,64] transposes into one tpv tile, one DVE copy out."""
            for rr in range(2):
                lh = 2 * t + rr
                hsl = slice(rr * 64, rr * 64 + 64)
                tpv = ps.tile(
                    [128, 256], mmdt, tag=tag[rr], name="tpv", bufs=1
                )
                for jj in range(4):
                    j = 4 * sc + jj
                    nc.tensor.transpose(
                        tpv[:, jj * 64 : (jj + 1) * 64],
                        q_sb[t][hsl, j * 128 : (j + 1) * 128],
                        idb_t[hsl, 0:64],
                    )
                vv = vaug[lh][:].rearrange("p (c w) -> p c w", w=65)
                nc.vector.tensor_copy(
                    vv[:, 4 * sc : 4 * sc + 4, 0:64],
                    tpv[:].rearrange("p (c w) -> p c w", w=64),
                )

        def emit_drain_chunk(prev_state, kc_rev, immediate=False):
            qc_p, t_p, ebuf_p, ctxs_p = prev_state
            for rr in range(2):
                nc.tensor.matmul(
                    ctxs_p[rr][:],
                    lhsT=vaug[2 * t_p + rr][:, kc_rev * 65 : kc_rev * 65 + 65],
                    rhs=ebuf_p[:, kc_rev * 1024 + rr * 512 : kc_rev * 1024 + rr * 512 + 512],
                    start=(kc_rev == (0 if immediate else NKC - 1)),
                    stop=(kc_rev == (NKC - 1 if immediate else 0)),
                )

        def emit_finalize(prev_state):
            qc_p, t_p, ebuf_p, ctxs_p = prev_state
            qsl_p = slice(qc_p * 512, (qc_p + 1) * 512)
            for rr in range(2):
                lh = 2 * t_p + rr
                cs = csp.tile([128, 512], f32, tag="cs")
                nc.vector.tensor_copy(cs[0:65, :], ctxs_p[rr][:])
                # one 65-row DMA (ctx + denominator) on the sync ring, which
                # is idle through the stream phase
                nc.sync.dma_start(out[lh * 65 : lh * 65 + 65, qsl_p], cs[0:65, :])

        # ---------- schedule ----------
        # Phase P: all projections, t-major (t=0 fully first).  Once kk(0)
        # exists, stream 0's first eight exp units weave between the t=1
        # projection chunks, filling ACT's otherwise-idle projection window
        # with early score exps.  v_aug transposes for a chunk are emitted
        # one iteration later, so the PE never waits on the DVE q-add that
        # produces their input.
        emit_vaug_ones(0)
        emit_vaug_ones(1)

        streams = [(qc, t) for t in range(2) for qc in range(NSC)]
        NS = len(streams)
        states = [None] * NS
        NG = NKC // 3  # 5 full 3-kc groups; kc=15 rides alone as a tail unit
        NU = 2 * NG + 1  # 11 emission units per stream

        def emit_score(s, kc, rr, dst):
            qc, t = streams[s]
            nc.tensor.matmul(
                dst,
                lhsT=kk_sb[t][rr * 64 : rr * 64 + 64, kc * 128 : (kc + 1) * 128],
                rhs=q_sb[t][rr * 64 : rr * 64 + 64, qc * 512 : (qc + 1) * 512],
                start=True,
                stop=True,
            )

        def emit_unit(s, u):
            if u == 0:
                # ctx tiles are allocated lazily at the stream's first drain
                # (mid next stream), so they never overlap the tpv transposes
                # sharing the cA/cB banks during the projection phase.
                states[s] = (streams[s][0], streams[s][1], etb[s % 3], None)
            ebuf = states[s][2]
            sp = ps.tile([128, 1536], f32, tag="sp", name="sp", bufs=2)
            if u == NU - 1:  # tail: kc = 15 alone, 1024-wide exp
                kc = NKC - 1
                emit_score(s, kc, 0, sp[:, 0:512])
                emit_score(s, kc, 1, sp[:, 512:1024])
                nc.scalar.activation(
                    ebuf[:, kc * 1024 : (kc + 1) * 1024],
                    sp[:, 0:1024],
                    AF.Exp,
                    scale=0.125,
                )
                return
            g, half = u // 2, u % 2
            if half == 0:
                emit_score(s, 3 * g, 0, sp[:, 0:512])
                emit_score(s, 3 * g, 1, sp[:, 512:1024])
                emit_score(s, 3 * g + 1, 0, sp[:, 1024:1536])
                base = 3 * g * 1024
            else:
                emit_score(s, 3 * g + 1, 1, sp[:, 0:512])
                emit_score(s, 3 * g + 2, 0, sp[:, 512:1024])
                emit_score(s, 3 * g + 2, 1, sp[:, 1024:1536])
                base = 3 * g * 1024 + 1536
            nc.scalar.activation(
                ebuf[:, base : base + 1536], sp[:], AF.Exp, scale=0.125
            )

        def emit_drain_for(n):
            s, kc = n // NKC, n % NKC
            if kc == 0:
                qc, t, ebuf, _ = states[s]
                states[s] = (
                    qc,
                    t,
                    ebuf,
                    [
                        ps.tile([65, 512], f32, tag="cA", name="ctxA", bufs=1),
                        ps.tile([65, 512], f32, tag="cB", name="ctxB", bufs=1),
                    ],
                )
            emit_drain_chunk(states[s], kc, immediate=True)
            if kc == NKC - 1:
                emit_finalize(states[s])

        def unit_exped(kc):
            if kc == NKC - 1:
                return NU - 1
            g, r = kc // 3, kc % 3
            return 2 * g if r == 0 else 2 * g + 1


        total_units = NS * NU
        due = [[] for _ in range(total_units + 1)]
        for s in range(NS):
            for kc in range(NKC):
                m = s * NKC + kc
                if s < NS - 1:
                    # drain during the next stream, spread one pair per unit
                    gu = (s + 1) * NU + (kc * NU) // NKC
                else:
                    # final stream: short 2-unit delay for a short tail
                    gu = s * NU + unit_exped(kc) + 2
                due[min(gu, total_units)].append(m)

        order = [(sc, t) for t in range(2) for sc in range(NSC)]
        for it, (sc, t) in enumerate(order):
            if it == 4:
                emit_kk(0)
            emit_proj_chunk(t, sc)
            if it >= 1:
                psc, pt = order[it - 1]
                emit_vaug_chunk(pt, psc, tag=("cA", "cB"))
            if it >= 4:
                emit_unit(0, 2 * (it - 4))
                emit_unit(0, 2 * (it - 4) + 1)
        emit_kk(1)
        emit_vaug_chunk(1, 3, tag=("cA", "cB"))

        # Phase S: continue the flat unit pipeline (stream 0's first eight
        # units were already emitted during the projection weave).
        for U in range(total_units):
            s, u = U // NU, U % NU
            if not (s == 0 and u < 8):
                emit_unit(s, u)
            for m in due[U]:
                emit_drain_for(m)
        for m in due[total_units]:
            emit_drain_for(m)

    nc.compile()
    return nc


def kernel(hidden_states, attention_mask, Wq, bq, Wk, bk):
    global _compiled, LAST_RESULT
    hs = np.asarray(hidden_states, dtype=np.float32)
    am = np.asarray(attention_mask)
    Wq = np.asarray(Wq, dtype=np.float32)
    Wk = np.asarray(Wk, dtype=np.float32)
    bq = np.asarray(bq, dtype=np.float32)
    bk = np.asarray(bk, dtype=np.float32)

    if _compiled is None:
        _compiled = _build()
    nc = _compiled

    from concourse.bass_utils import run_bass_kernel_spmd

    if MM_DTYPE == "bf16":
        import ml_dtypes

        def to_mmdt(x):
            return np.ascontiguousarray(
                np.asarray(x, np.float32).astype(ml_dtypes.bfloat16)
            )

    elif MM_DTYPE == "f32r":

        def to_mmdt(x):
            # fp32r = 1s/8e/11m (top 20 bits of fp32), round-to-nearest-even
            b = np.ascontiguousarray(x, dtype=np.float32).view(np.uint32)
            lsb = (b >> np.uint32(12)) & np.uint32(1)
            r = (b + np.uint32(0x7FF) + lsb) & np.uint32(0xFFFFF000)
            return r.view(np.float32)

    else:

        def to_mmdt(x):
            return np.ascontiguousarray(x, dtype=np.float32)

    idb = to_mmdt(np.tile(np.eye(64, dtype=np.float32), (2, 1)))
    in_maps = []
    for c in range(NCORES):
        b, g = c // HG, c % HG
        cols = slice(g * CPG, (g + 1) * CPG)
        bq_s = bq[cols].reshape(2, 128).T
        bk_s = bk[cols].reshape(2, 128).T
        smalls = np.concatenate(
            [bq_s, -bq_s, -(bq_s + bk_s), np.ones((128, 16), np.float32)],
            axis=1,
        ).astype(np.float32)
        in_maps.append(
            {
                "ht": to_mmdt(hs[b].T),
                "wall": to_mmdt(
                    np.concatenate(
                        [Wq[:, cols], Wq[:, cols] + Wk[:, cols]],
                        axis=1,
                    )
                ),
                "smalls": np.ascontiguousarray(smalls),
                "idb": idb,
            }
        )

    res = run_bass_kernel_spmd(nc, in_maps, list(range(NCORES)))
    LAST_RESULT = res

    outp = np.empty((B, S, H * DH), dtype=np.float32)
    for c in range(NCORES):
        b, g = c // HG, c % HG
        raw = res.results[c]["out"].reshape(4, 65, S)  # per head: 64 ctx + den
        ctxT = raw[:, 0:64, :] / raw[:, 64:65, :]
        outp[b, :, g * CPG : (g + 1) * CPG] = ctxT.reshape(CPG, S).T

    # attention_mask==0 masks whole query rows -> uniform probs -> ctx row is
    # the mean of q over all key positions. Never triggers for all-ones masks.
    if (am == 0).any():
        for b in range(B):
            rows = np.nonzero(am[b] == 0)[0]
            if rows.size:
                q_full = hs[b] @ Wq + bq
                outp[b, rows, :] = q_full.mean(axis=0)
    return outp
